# revision 9
# baseline (speedup 1.0000x reference)
"""APPNP GNN kernel for 8 TRN2 NeuronCores (v2).

Strategy (vertex-cut / dst-partitioned):
  - Nodes row-sharded across 8 cores (12500 rows/core, padded to 12544).
  - MLP encoder data-parallel per core (features host-transposed).
  - K=10 propagation steps; each step:
      AllGather of norm-scaled h (bf16, rows padded to 256B),
      dma_gather of source rows for this core's in-edges (4 src chunks so
      row indices fit int16; tokens padded per (supergroup-of-8-windows,
      chunk) to a multiple of 128 -> ~2-4% padding),
      segment-sum via one-hot stationary matmuls accumulating in PSUM
      (128-dst-node windows; one-hots precomputed on host, streamed from
      DRAM), epilogue fuses norm_dst scale + alpha*h0 + norm_src scale.
  - dma_scatter_add is NOT used (HBM CCE RMW loses updates on duplicate
    indices).
"""

import sys

sys.path.insert(0, "/opt/trn_rl_repo")

import dataclasses
import numpy as np
import ml_dtypes

import concourse.bass as bass
import concourse.mybir as mybir
import concourse.tile as tile
from concourse import bacc
from concourse.bass_utils import run_bass_kernel_spmd

BF16 = ml_dtypes.bfloat16


@dataclasses.dataclass(frozen=True)
class Cfg:
    NC: int = 8          # cores
    N: int = 100000      # nodes
    IN: int = 512        # input feature dim
    HD: int = 64         # hidden dim
    OUT: int = 64        # output dim
    K: int = 10          # propagation steps
    ALPHA: float = 0.1
    NCHUNK: int = 4      # gather source chunks (int16 idx range)
    CALL_TILES: int = 32  # max tiles (128 tokens each) per dma_gather call
    SG: int = 8          # windows per supergroup (= windows per psum bank)
    BSG: int = 3         # supergroups per batch
    H0W: int = 48        # windows in AG half 0 (must be a batch boundary)

    @property
    def NSH(self):
        return self.N // self.NC

    @property
    def NT(self):
        return (self.NSH + 127) // 128  # windows per core

    @property
    def NPAD(self):
        return self.NT * 128

    @property
    def GROWS(self):
        return self.NC * self.NPAD

    @property
    def H0R(self):
        return self.H0W * 128

    @property
    def H1R(self):
        return self.NPAD - self.H0R

    @property
    def CROWS0(self):
        return self.NC * self.H0R // 2

    @property
    def CROWS1(self):
        return self.NC * self.H1R // 2

    @property
    def supergroups(self):
        out = []
        w = 0
        while w < self.NT:
            out.append((w, min(w + self.SG, self.NT)))
            w += self.SG
        return out

    @property
    def batches(self):
        """Batches of supergroups: list of (ws, we, [sg indices])."""
        sgs = self.supergroups
        out = []
        i = 0
        while i < len(sgs):
            j = min(i + self.BSG, len(sgs))
            if len(sgs) - j == 1:  # avoid lone trailing supergroup
                j = len(sgs)
            out.append((sgs[i][0], sgs[j - 1][1], list(range(i, j))))
            i = j
        return out


CFG = Cfg()


def _preprocess(edge_index, cfg: Cfg):
    src = np.asarray(edge_index[0], dtype=np.int64)
    dst = np.asarray(edge_index[1], dtype=np.int64)

    deg_out = np.bincount(src, minlength=cfg.N).astype(np.float32)
    deg_in = np.bincount(dst, minlength=cfg.N).astype(np.float32)
    norm_src = 1.0 / np.sqrt(np.clip(deg_out, 1.0, None))
    norm_dst = 1.0 / np.sqrt(np.clip(deg_in, 1.0, None))

    core = dst // cfg.NSH
    dst_local = dst - core * cfg.NSH
    w = dst_local // 128
    slot = dst_local % 128
    sg = w // cfg.SG
    d = src // cfg.NSH
    j = src % cfg.NSH
    h1 = j >= cfg.H0R
    g0 = d * cfg.H0R + j
    g1 = d * cfg.H1R + (j - cfg.H0R)
    q = np.where(h1, 2 + g1 // cfg.CROWS1, g0 // cfg.CROWS0)
    lidx = np.where(h1, g1 % cfg.CROWS1, g0 % cfg.CROWS0)

    NSG = len(cfg.supergroups)
    key = (core * NSG + sg) * cfg.NCHUNK + q
    counts = np.bincount(key, minlength=cfg.NC * NSG * cfg.NCHUNK).reshape(
        cfg.NC, NSG, cfg.NCHUNK
    )
    P = -(-counts.max(axis=0) // 128)  # [NSG, NCHUNK] tiles per (sg, q)

    # static tile stream: per batch, per chunk, supergroups ascending
    tiles = []          # (sg, q)
    calls = []          # (q, tile_off, ntiles)
    grp_tile0 = {}      # (sg, q) -> first tile index
    for (_, _, sgl) in cfg.batches:
        for qq in range(cfg.NCHUNK):
            group = []
            for si in sgl:
                grp_tile0[(si, qq)] = len(tiles) + len(group)
                group += [(si, qq)] * int(P[si, qq])
            off = 0
            while off < len(group):
                n = min(cfg.CALL_TILES, len(group) - off)
                calls.append((qq, len(tiles) + off, n))
                off += n
            tiles.extend(group)
    TOT = len(tiles)

    # per-core token placement: edges sorted by (core, sg, q, w, slot)
    sort_key = (((core * NSG + sg) * cfg.NCHUNK + q) * cfg.NT + w) * 128 + slot
    order = np.argsort(sort_key, kind="stable")
    s_core, s_sg, s_q = core[order], sg[order], q[order]
    s_w, s_slot, s_lidx = w[order], slot[order], lidx[order]

    gkey = (s_core * NSG + s_sg) * cfg.NCHUNK + s_q
    gstarts = np.searchsorted(gkey, np.arange(cfg.NC * NSG * cfg.NCHUNK))
    gends = np.searchsorted(gkey, np.arange(cfg.NC * NSG * cfg.NCHUNK) + 1)

    tile_windows = [set() for _ in range(TOT)]
    core_tok = []  # per core: (tile, part, w, slot, lidx) arrays
    for c in range(cfg.NC):
        tl_l, pp_l, ww_l, sl_l, li_l = [], [], [], [], []
        for si in range(NSG):
            for qq in range(cfg.NCHUNK):
                gi = (c * NSG + si) * cfg.NCHUNK + qq
                a, b = gstarts[gi], gends[gi]
                if b == a:
                    continue
                t0 = grp_tile0[(si, qq)]
                j = np.arange(b - a)
                tl_l.append(t0 + j // 128)
                pp_l.append(j % 128)
                ww_l.append(s_w[a:b])
                sl_l.append(s_slot[a:b])
                li_l.append(s_lidx[a:b])
        tl = np.concatenate(tl_l)
        ww = np.concatenate(ww_l)
        core_tok.append(
            (tl, np.concatenate(pp_l), ww, np.concatenate(sl_l), np.concatenate(li_l))
        )
        # distinct (tile, window) incidences for this core
        tw = np.unique(tl.astype(np.int64) * cfg.NT + ww)
        for v in tw:
            tile_windows[int(v) // cfg.NT].add(int(v) % cfg.NT)

    pairs = []
    pair_idx = {}
    for t in range(TOT):
        for wv in sorted(tile_windows[t]):
            pair_idx[(t, wv)] = len(pairs)
            pairs.append((t, wv))
    NPAIR = len(pairs)

    call_pairs = []
    for (qq, t0, nt) in calls:
        idxs = [
            pair_idx[(t, wv)] for t in range(t0, t0 + nt) for wv in tile_windows[t]
        ]
        call_pairs.append((min(idxs) if idxs else 0, len(idxs)))

    per_core = []
    for c in range(cfg.NC):
        tl, pp, ww, sl, li = core_tok[c]
        g_tok = np.zeros(TOT * 128, dtype=np.int16)
        g_tok[tl * 128 + pp] = li.astype(np.int16)
        cols = []
        for (qq, t0, nt) in calls:
            seg = g_tok[t0 * 128 : (t0 + nt) * 128]
            cols.append(np.tile(seg.reshape(-1, 16).T, (8, 1)))
        gidx = np.ascontiguousarray(np.concatenate(cols, axis=1)).astype(np.int16)

        pj = np.array(
            [pair_idx[(int(t), int(wv))] for t, wv in zip(tl, ww)], dtype=np.int64
        )
        oh = np.zeros((128, NPAIR * 128), dtype=BF16)
        oh[pp, pj * 128 + sl] = BF16(1.0)
        per_core.append((gidx, oh))

    sched = {
        "P": P,
        "tiles": tiles,
        "calls": calls,
        "call_pairs": call_pairs,
        "pairs": pairs,
        "TOT": TOT,
        "NPAIR": NPAIR,
        "pairs_per_w": np.bincount(
            np.array([wv for (_, wv) in pairs], dtype=np.int64), minlength=cfg.NT
        ),
    }
    return sched, per_core, norm_src, norm_dst


def _build_nc(cfg: Cfg, sched):
    f32 = mybir.dt.float32
    bf16 = mybir.dt.bfloat16
    i16 = mybir.dt.int16
    TOT = sched["TOT"]
    tiles = sched["tiles"]
    calls = sched["calls"]
    call_pairs = sched["call_pairs"]
    pairs = sched["pairs"]
    NPAIR = sched["NPAIR"]
    ppw = sched["pairs_per_w"]

    nc = bacc.Bacc(
        "TRN2",
        target_bir_lowering=False,
        debug=False,
        num_devices=cfg.NC,
        num_swdge_queues=4,
        dynamic_dma_scratch_size=32768,
    )
    ft_d = nc.dram_tensor("ft", [cfg.IN, cfg.NPAD], f32, kind="ExternalInput")
    w1_d = nc.dram_tensor("w1", [cfg.IN, cfg.HD], f32, kind="ExternalInput")
    b1_d = nc.dram_tensor("b1c", [cfg.HD, 1], f32, kind="ExternalInput")
    w2_d = nc.dram_tensor("w2", [cfg.HD, cfg.OUT], f32, kind="ExternalInput")
    b2_d = nc.dram_tensor("b2bc", [128, cfg.OUT], f32, kind="ExternalInput")
    nsrc_d = nc.dram_tensor("nsrc", [128, cfg.NT], f32, kind="ExternalInput")
    ndst_d = nc.dram_tensor("ndst09", [128, cfg.NT], f32, kind="ExternalInput")
    gidx_d = nc.dram_tensor("gidx", [128, TOT * 8], i16, kind="ExternalInput")
    oh_d = nc.dram_tensor("oh", [128, NPAIR * 128], bf16, kind="ExternalInput")
    out_d = nc.dram_tensor("out", [cfg.NPAD, cfg.OUT], f32, kind="ExternalOutput")

    KIN = cfg.IN // 128
    oh_v = oh_d.rearrange("p (n i) -> p n i", i=128)

    with tile.TileContext(nc) as tc:
        with (
            tc.tile_pool(name="const", bufs=1) as constp,
            tc.tile_pool(name="dram", bufs=1, space="DRAM") as dramp,
            tc.tile_pool(name="mlpio", bufs=3) as mlpio,
            tc.tile_pool(name="msgp", bufs=3) as msgp,
            tc.tile_pool(name="idxp", bufs=3) as idxp,
            tc.tile_pool(name="ohp", bufs=2) as ohp,
            tc.tile_pool(name="stagep", bufs=2) as stagep,
            tc.tile_pool(name="epp", bufs=8) as epp,
            tc.tile_pool(name="psum", bufs=8, space="PSUM") as psump,
        ):
            hs_b0s = [
                dramp.tile([cfg.H0R, 128], bf16, name=f"hsb0_{j}") for j in range(cfg.K)
            ]
            hs_b1s = [
                dramp.tile([cfg.H1R, 128], bf16, name=f"hsb1_{j}") for j in range(cfg.K)
            ]
            hs_f0s = [
                dramp.tile([cfg.NC * cfg.H0R, 128], bf16, addr_space="Shared", name=f"hsf0_{j}")
                for j in range(cfg.K)
            ]
            hs_f1s = [
                dramp.tile([cfg.NC * cfg.H1R, 128], bf16, addr_space="Shared", name=f"hsf1_{j}")
                for j in range(cfg.K)
            ]
            hs_b0_vs = [hb.rearrange("(n p) c -> p n c", p=128) for hb in hs_b0s]
            hs_b1_vs = [hb.rearrange("(n p) c -> p n c", p=128) for hb in hs_b1s]

            def hs_window_view(it, i):
                if i < cfg.H0W:
                    return hs_b0_vs[it][:, i, :]
                return hs_b1_vs[it][:, i - cfg.H0W, :]

            def chunk_src(it, qq):
                if qq < 2:
                    return hs_f0s[it][qq * cfg.CROWS0 : (qq + 1) * cfg.CROWS0, :]
                return hs_f1s[it][(qq - 2) * cfg.CROWS1 : (qq - 1) * cfg.CROWS1, :]

            out_v = out_d.rearrange("(n p) c -> p n c", p=128)

            # ---- resident constants ----
            w1_s = constp.tile([128, KIN, cfg.HD], f32)
            nc.sync.dma_start(w1_s[:], w1_d.rearrange("(k p) h -> p k h", p=128)[:, :, :])
            b1_s = constp.tile([cfg.HD, 1], f32)
            nc.sync.dma_start(b1_s[:], b1_d[:])
            w2_s = constp.tile([cfg.HD, cfg.OUT], f32)
            nc.sync.dma_start(w2_s[:], w2_d[:])
            b2_s = constp.tile([128, cfg.OUT], f32)
            nc.sync.dma_start(b2_s[:], b2_d[:])
            nsrc_s = constp.tile([128, cfg.NT], f32)
            nc.sync.dma_start(nsrc_s[:], nsrc_d[:])
            ndst_s = constp.tile([128, cfg.NT], f32)
            nc.sync.dma_start(ndst_s[:], ndst_d[:])
            zero512 = constp.tile([128, 512], bf16)
            nc.vector.memset(zero512[:], 0)
            alpha_h0 = constp.tile([128, cfg.NT, cfg.OUT], f32)

            # ---- MLP encoder: h0 = relu(F @ W1 + b1) @ W2 + b2 ----
            ft_v = ft_d.rearrange("(k p) c -> p k c", p=128)
            for i in range(cfg.NT):
                ft_t = mlpio.tile([128, KIN, 128], f32, tag="ft")
                nc.sync.dma_start(ft_t[:], ft_v[:, :, i * 128 : (i + 1) * 128])
                p_gt = psump.tile([cfg.HD, 128], f32, tag="pp")
                for k in range(KIN):
                    nc.tensor.matmul(
                        p_gt[:],
                        w1_s[:, k, :],
                        ft_t[:, k, :],
                        start=(k == 0),
                        stop=(k == KIN - 1),
                    )
                gt = mlpio.tile([cfg.HD, 128], f32, tag="gt")
                nc.scalar.activation(
                    gt[:], p_gt[:], mybir.ActivationFunctionType.Relu, bias=b1_s[:, 0:1]
                )
                p_h0 = psump.tile([128, cfg.OUT], f32, tag="pp")
                nc.tensor.matmul(p_h0[:], gt[:], w2_s[:], start=True, stop=True)
                h0_t = mlpio.tile([128, cfg.OUT], f32, tag="h0")
                nc.vector.tensor_add(h0_t[:], p_h0[:], b2_s[:])
                nc.scalar.activation(
                    alpha_h0[:, i, :],
                    h0_t[:],
                    mybir.ActivationFunctionType.Copy,
                    scale=cfg.ALPHA,
                )
                hs_t = mlpio.tile([128, 128], bf16, tag="hs")
                nc.vector.memset(hs_t[:, cfg.OUT :], 0)
                nc.vector.tensor_scalar_mul(hs_t[:, 0 : cfg.OUT], h0_t[:], nsrc_s[:, i : i + 1])
                nc.sync.dma_start(hs_window_view(0, i), hs_t[:])

            # ---- K propagation steps ----
            for it in range(cfg.K):
                nc.gpsimd.collective_compute(
                    "AllGather",
                    mybir.AluOpType.bypass,
                    replica_groups=[list(range(cfg.NC))],
                    ins=[hs_b0s[it].opt()],
                    outs=[hs_f0s[it].opt()],
                )
                nc.gpsimd.collective_compute(
                    "AllGather",
                    mybir.AluOpType.bypass,
                    replica_groups=[list(range(cfg.NC))],
                    ins=[hs_b1s[it].opt()],
                    outs=[hs_f1s[it].opt()],
                )
                call_cursor = 0
                qcount = 0
                last = it == cfg.K - 1
                for bi, (ws, we, sgl) in enumerate(cfg.batches):
                    nbank = -(-(we - ws) // cfg.SG)
                    banks = [
                        psump.tile([128, 512], f32, tag="pp", name=f"pb{it}_{bi}_{j}")
                        for j in range(nbank)
                    ]
                    psum_map = {}
                    for ww in range(ws, we):
                        wi = ww - ws
                        psum_map[ww] = banks[wi // cfg.SG][
                            :, (wi % cfg.SG) * 64 : (wi % cfg.SG) * 64 + 64
                        ]
                    for bk in banks:
                        # start=True zeroes the whole 2KB zero-region (= one
                        # bank); do it once per bank so its windows accumulate
                        # with start=False.
                        nc.tensor.matmul(
                            bk[:], zero512[:, 0:128], zero512[:],
                            start=True, stop=False, skip_group_check=True,
                        )
                    while call_cursor < len(calls):
                        qq, t0, ntl = calls[call_cursor]
                        if tiles[t0][0] * cfg.SG >= we:
                            break
                        plo, pn = call_pairs[call_cursor]
                        call_cursor += 1
                        nidx = ntl * 128
                        gi = idxp.tile([128, cfg.CALL_TILES * 8], i16, tag="gi")
                        nc.sync.dma_start(
                            gi[:, : ntl * 8], gidx_d[:, t0 * 8 : (t0 + ntl) * 8]
                        )
                        msg = msgp.tile([128, cfg.CALL_TILES, 128], bf16, tag="msg")
                        nc.gpsimd.dma_gather(
                            msg[:, :ntl, :],
                            chunk_src(it, qq),
                            gi[:, : ntl * 8],
                            nidx,
                            nidx,
                            128,
                            single_packet=False,
                            queue_num=qcount % 4,
                        )
                        qcount += 1
                        oh = ohp.tile([128, cfg.CALL_TILES * 2, 128], bf16, tag="oh")
                        nc.sync.dma_start(oh[:, :pn, :], oh_v[:, plo : plo + pn, :])
                        for j in range(pn):
                            t, wv = pairs[plo + j]
                            nc.tensor.matmul(
                                psum_map[wv],
                                oh[:, j, :],
                                msg[:, t - t0, 0 : cfg.OUT],
                                start=False,
                                stop=False,
                                skip_group_check=True,
                            )
                    # epilogue for this batch
                    nw = we - ws
                    stg = stagep.tile(
                        [128, (cfg.BSG + 1) * cfg.SG, cfg.OUT], f32, tag="stgf",
                        name=f"sf{it}_{bi}",
                    )
                    for ww in range(ws, we):
                        wi = ww - ws
                        t1 = epp.tile([128, cfg.OUT], f32, tag="t1")
                        if ppw[ww] == 0:
                            nc.vector.memset(t1[:], 0.0)
                        else:
                            nc.scalar.activation(
                                t1[:],
                                psum_map[ww],
                                mybir.ActivationFunctionType.Copy,
                                scale=ndst_s[:, ww : ww + 1],
                            )
                        if last:
                            nc.vector.tensor_add(
                                stg[:, wi, :], t1[:], alpha_h0[:, ww, :]
                            )
                        else:
                            t2 = epp.tile([128, cfg.OUT], f32, tag="t2")
                            nc.vector.tensor_add(t2[:], t1[:], alpha_h0[:, ww, :])
                            nc.vector.tensor_scalar_mul(
                                stg[:, wi, :], t2[:], nsrc_s[:, ww : ww + 1]
                            )
                    if last:
                        nc.sync.dma_start(out_v[:, ws:we, :], stg[:, :nw, :])
                    else:
                        stb = stagep.tile(
                            [128, (cfg.BSG + 1) * cfg.SG, 128], bf16, tag="stgb",
                            name=f"sb{it}_{bi}",
                        )
                        nc.vector.memset(stb[:, :nw, cfg.OUT :], 0)
                        nc.scalar.activation(
                            stb[:, :nw, 0 : cfg.OUT],
                            stg[:, :nw, :],
                            mybir.ActivationFunctionType.Copy,
                        )
                        if we <= cfg.H0W:
                            nc.sync.dma_start(
                                hs_b0_vs[it + 1][:, ws:we, :], stb[:, :nw, :]
                            )
                        elif ws >= cfg.H0W:
                            nc.sync.dma_start(
                                hs_b1_vs[it + 1][:, ws - cfg.H0W : we - cfg.H0W, :],
                                stb[:, :nw, :],
                            )
                        else:
                            k0 = cfg.H0W - ws
                            nc.sync.dma_start(
                                hs_b0_vs[it + 1][:, ws : cfg.H0W, :], stb[:, :k0, :]
                            )
                            nc.sync.dma_start(
                                hs_b1_vs[it + 1][:, 0 : we - cfg.H0W, :],
                                stb[:, k0:nw, :],
                            )
    nc.compile()
    return nc


_CACHE = {}


def _get_compiled(cfg: Cfg, sched_sig, sched):
    key = (cfg, sched_sig)
    if key not in _CACHE:
        _CACHE[key] = _build_nc(cfg, sched)
    return _CACHE[key]


def kernel(features, edge_index, W1, b1, W2, b2):
    cfg = CFG
    features = np.asarray(features, dtype=np.float32)
    W1 = np.asarray(W1, dtype=np.float32)
    b1 = np.asarray(b1, dtype=np.float32)
    W2 = np.asarray(W2, dtype=np.float32)
    b2 = np.asarray(b2, dtype=np.float32)

    sched, per_core, norm_src, norm_dst = _preprocess(edge_index, cfg)
    sched_sig = (
        sched["TOT"],
        sched["NPAIR"],
        tuple(sched["calls"]),
        tuple(sched["pairs"]),
    )
    nc = _get_compiled(cfg, sched_sig, sched)

    b1c = b1.reshape(cfg.HD, 1)
    b2bc = np.ascontiguousarray(np.broadcast_to(b2[None, :], (128, cfg.OUT)))

    in_maps = []
    for c in range(cfg.NC):
        lo = c * cfg.NSH
        f = np.zeros((cfg.IN, cfg.NPAD), dtype=np.float32)
        f[:, : cfg.NSH] = features[lo : lo + cfg.NSH].T
        ns = np.ones(cfg.NPAD, dtype=np.float32)
        ns[: cfg.NSH] = norm_src[lo : lo + cfg.NSH]
        nd = np.ones(cfg.NPAD, dtype=np.float32)
        nd[: cfg.NSH] = norm_dst[lo : lo + cfg.NSH]
        gidx, oh = per_core[c]
        in_maps.append(
            {
                "ft": f,
                "w1": W1,
                "b1c": b1c,
                "w2": W2,
                "b2bc": b2bc,
                "nsrc": np.ascontiguousarray(ns.reshape(cfg.NT, 128).T),
                "ndst09": np.ascontiguousarray(
                    ((1.0 - cfg.ALPHA) * nd).reshape(cfg.NT, 128).T
                ),
                "gidx": gidx,
                "oh": oh,
            }
        )

    res = run_bass_kernel_spmd(nc, in_maps, list(range(cfg.NC)))
    out = np.concatenate(
        [res.results[c]["out"][: cfg.NSH] for c in range(cfg.NC)], axis=0
    )
    return out


# revision 10
# speedup vs baseline: 1.0226x; 1.0226x over previous
"""APPNP GNN kernel for 8 TRN2 NeuronCores (v2).

Strategy (vertex-cut / dst-partitioned):
  - Nodes row-sharded across 8 cores (12500 rows/core, padded to 12544).
  - MLP encoder data-parallel per core (features host-transposed).
  - K=10 propagation steps; each step:
      AllGather of norm-scaled h (bf16, rows padded to 256B),
      dma_gather of source rows for this core's in-edges (4 src chunks so
      row indices fit int16; tokens padded per (supergroup-of-8-windows,
      chunk) to a multiple of 128 -> ~2-4% padding),
      segment-sum via one-hot stationary matmuls accumulating in PSUM
      (128-dst-node windows; one-hots precomputed on host, streamed from
      DRAM), epilogue fuses norm_dst scale + alpha*h0 + norm_src scale.
  - dma_scatter_add is NOT used (HBM CCE RMW loses updates on duplicate
    indices).
"""

import sys

sys.path.insert(0, "/opt/trn_rl_repo")

import dataclasses
import numpy as np
import ml_dtypes

import concourse.bass as bass
import concourse.mybir as mybir
import concourse.tile as tile
from concourse import bacc
from concourse.bass_utils import run_bass_kernel_spmd

BF16 = ml_dtypes.bfloat16


@dataclasses.dataclass(frozen=True)
class Cfg:
    NC: int = 8          # cores
    N: int = 100000      # nodes
    IN: int = 512        # input feature dim
    HD: int = 64         # hidden dim
    OUT: int = 64        # output dim
    K: int = 10          # propagation steps
    ALPHA: float = 0.1
    NCHUNK: int = 4      # gather source chunks (int16 idx range)
    CALL_TILES: int = 32  # max tiles (128 tokens each) per dma_gather call
    SG: int = 8          # windows per supergroup (= windows per psum bank)
    BSG: int = 3         # supergroups per batch
    H0W: int = 48        # windows in AG half 0 (must be a batch boundary)

    @property
    def NSH(self):
        return self.N // self.NC

    @property
    def NT(self):
        return (self.NSH + 127) // 128  # windows per core

    @property
    def NPAD(self):
        return self.NT * 128

    @property
    def GROWS(self):
        return self.NC * self.NPAD

    @property
    def H0R(self):
        return self.H0W * 128

    @property
    def H1R(self):
        return self.NPAD - self.H0R

    @property
    def CROWS0(self):
        return self.NC * self.H0R // 2

    @property
    def CROWS1(self):
        return self.NC * self.H1R // 2

    @property
    def supergroups(self):
        out = []
        w = 0
        while w < self.NT:
            out.append((w, min(w + self.SG, self.NT)))
            w += self.SG
        return out

    @property
    def batches(self):
        """Batches of supergroups: list of (ws, we, [sg indices])."""
        sgs = self.supergroups
        out = []
        i = 0
        while i < len(sgs):
            j = min(i + self.BSG, len(sgs))
            if len(sgs) - j == 1:  # avoid lone trailing supergroup
                j = len(sgs)
            out.append((sgs[i][0], sgs[j - 1][1], list(range(i, j))))
            i = j
        return out


CFG = Cfg()


def _preprocess(edge_index, cfg: Cfg):
    src = np.asarray(edge_index[0], dtype=np.int64)
    dst = np.asarray(edge_index[1], dtype=np.int64)

    deg_out = np.bincount(src, minlength=cfg.N).astype(np.float32)
    deg_in = np.bincount(dst, minlength=cfg.N).astype(np.float32)
    norm_src = 1.0 / np.sqrt(np.clip(deg_out, 1.0, None))
    norm_dst = 1.0 / np.sqrt(np.clip(deg_in, 1.0, None))

    core = dst // cfg.NSH
    dst_local = dst - core * cfg.NSH
    w = dst_local // 128
    slot = dst_local % 128
    sg = w // cfg.SG
    d = src // cfg.NSH
    j = src % cfg.NSH
    h1 = j >= cfg.H0R
    g0 = d * cfg.H0R + j
    g1 = d * cfg.H1R + (j - cfg.H0R)
    q = np.where(h1, 2 + g1 // cfg.CROWS1, g0 // cfg.CROWS0)
    lidx = np.where(h1, g1 % cfg.CROWS1, g0 % cfg.CROWS0)

    NSG = len(cfg.supergroups)
    key = (core * NSG + sg) * cfg.NCHUNK + q
    counts = np.bincount(key, minlength=cfg.NC * NSG * cfg.NCHUNK).reshape(
        cfg.NC, NSG, cfg.NCHUNK
    )
    P = -(-counts.max(axis=0) // 128)  # [NSG, NCHUNK] tiles per (sg, q)

    # static tile stream: per batch, per chunk, supergroups ascending
    tiles = []          # (sg, q)
    calls = []          # (q, tile_off, ntiles)
    grp_tile0 = {}      # (sg, q) -> first tile index
    for (_, _, sgl) in cfg.batches:
        for qq in range(cfg.NCHUNK):
            group = []
            for si in sgl:
                grp_tile0[(si, qq)] = len(tiles) + len(group)
                group += [(si, qq)] * int(P[si, qq])
            off = 0
            while off < len(group):
                n = min(cfg.CALL_TILES, len(group) - off)
                calls.append((qq, len(tiles) + off, n))
                off += n
            tiles.extend(group)
    TOT = len(tiles)

    # per-core token placement: edges sorted by (core, sg, q, w, slot)
    sort_key = (((core * NSG + sg) * cfg.NCHUNK + q) * cfg.NT + w) * 128 + slot
    order = np.argsort(sort_key, kind="stable")
    s_core, s_sg, s_q = core[order], sg[order], q[order]
    s_w, s_slot, s_lidx = w[order], slot[order], lidx[order]

    gkey = (s_core * NSG + s_sg) * cfg.NCHUNK + s_q
    gstarts = np.searchsorted(gkey, np.arange(cfg.NC * NSG * cfg.NCHUNK))
    gends = np.searchsorted(gkey, np.arange(cfg.NC * NSG * cfg.NCHUNK) + 1)

    tile_windows = [set() for _ in range(TOT)]
    core_tok = []  # per core: (tile, part, w, slot, lidx) arrays
    for c in range(cfg.NC):
        tl_l, pp_l, ww_l, sl_l, li_l = [], [], [], [], []
        for si in range(NSG):
            for qq in range(cfg.NCHUNK):
                gi = (c * NSG + si) * cfg.NCHUNK + qq
                a, b = gstarts[gi], gends[gi]
                if b == a:
                    continue
                t0 = grp_tile0[(si, qq)]
                j = np.arange(b - a)
                tl_l.append(t0 + j // 128)
                pp_l.append(j % 128)
                ww_l.append(s_w[a:b])
                sl_l.append(s_slot[a:b])
                li_l.append(s_lidx[a:b])
        tl = np.concatenate(tl_l)
        ww = np.concatenate(ww_l)
        core_tok.append(
            (tl, np.concatenate(pp_l), ww, np.concatenate(sl_l), np.concatenate(li_l))
        )
        # distinct (tile, window) incidences for this core
        tw = np.unique(tl.astype(np.int64) * cfg.NT + ww)
        for v in tw:
            tile_windows[int(v) // cfg.NT].add(int(v) % cfg.NT)

    pairs = []
    pair_idx = {}
    for t in range(TOT):
        for wv in sorted(tile_windows[t]):
            pair_idx[(t, wv)] = len(pairs)
            pairs.append((t, wv))
    NPAIR = len(pairs)

    call_pairs = []
    for (qq, t0, nt) in calls:
        idxs = [
            pair_idx[(t, wv)] for t in range(t0, t0 + nt) for wv in tile_windows[t]
        ]
        call_pairs.append((min(idxs) if idxs else 0, len(idxs)))

    per_core = []
    for c in range(cfg.NC):
        tl, pp, ww, sl, li = core_tok[c]
        g_tok = np.zeros(TOT * 128, dtype=np.int16)
        g_tok[tl * 128 + pp] = li.astype(np.int16)
        cols = []
        for (qq, t0, nt) in calls:
            seg = g_tok[t0 * 128 : (t0 + nt) * 128]
            cols.append(np.tile(seg.reshape(-1, 16).T, (8, 1)))
        gidx = np.ascontiguousarray(np.concatenate(cols, axis=1)).astype(np.int16)

        pj = np.array(
            [pair_idx[(int(t), int(wv))] for t, wv in zip(tl, ww)], dtype=np.int64
        )
        oh = np.zeros((128, NPAIR * 128), dtype=BF16)
        oh[pp, pj * 128 + sl] = BF16(1.0)
        per_core.append((gidx, oh))

    sched = {
        "P": P,
        "tiles": tiles,
        "calls": calls,
        "call_pairs": call_pairs,
        "pairs": pairs,
        "TOT": TOT,
        "NPAIR": NPAIR,
        "pairs_per_w": np.bincount(
            np.array([wv for (_, wv) in pairs], dtype=np.int64), minlength=cfg.NT
        ),
    }
    return sched, per_core, norm_src, norm_dst


def _build_nc(cfg: Cfg, sched):
    f32 = mybir.dt.float32
    bf16 = mybir.dt.bfloat16
    i16 = mybir.dt.int16
    TOT = sched["TOT"]
    tiles = sched["tiles"]
    calls = sched["calls"]
    call_pairs = sched["call_pairs"]
    pairs = sched["pairs"]
    NPAIR = sched["NPAIR"]
    ppw = sched["pairs_per_w"]

    nc = bacc.Bacc(
        "TRN2",
        target_bir_lowering=False,
        debug=False,
        num_devices=cfg.NC,
        num_swdge_queues=4,
        dynamic_dma_scratch_size=32768,
    )
    ft_d = nc.dram_tensor("ft", [cfg.IN, cfg.NPAD], f32, kind="ExternalInput")
    w1_d = nc.dram_tensor("w1", [cfg.IN, cfg.HD], f32, kind="ExternalInput")
    b1_d = nc.dram_tensor("b1c", [cfg.HD, 1], f32, kind="ExternalInput")
    w2_d = nc.dram_tensor("w2", [cfg.HD, cfg.OUT], f32, kind="ExternalInput")
    b2_d = nc.dram_tensor("b2bc", [128, cfg.OUT], f32, kind="ExternalInput")
    nsrc_d = nc.dram_tensor("nsrc", [128, cfg.NT], f32, kind="ExternalInput")
    ndst_d = nc.dram_tensor("ndst09", [128, cfg.NT], f32, kind="ExternalInput")
    gidx_d = nc.dram_tensor("gidx", [128, TOT * 8], i16, kind="ExternalInput")
    oh_d = nc.dram_tensor("oh", [128, NPAIR * 128], bf16, kind="ExternalInput")
    out_d = nc.dram_tensor("out", [cfg.NPAD, cfg.OUT], f32, kind="ExternalOutput")

    KIN = cfg.IN // 128
    oh_v = oh_d.rearrange("p (n i) -> p n i", i=128)
    max_nw = max(we - ws for (ws, we, _) in cfg.batches)

    with tile.TileContext(nc) as tc:
        with (
            tc.tile_pool(name="const", bufs=1) as constp,
            tc.tile_pool(name="dram", bufs=1, space="DRAM") as dramp,
            tc.tile_pool(name="mlpio", bufs=3) as mlpio,
            tc.tile_pool(name="msgp", bufs=4) as msgp,
            tc.tile_pool(name="idxp", bufs=3) as idxp,
            tc.tile_pool(name="ohp", bufs=3) as ohp,
            tc.tile_pool(name="stagep", bufs=2) as stagep,
            tc.tile_pool(name="epp", bufs=8) as epp,
            tc.tile_pool(name="psum", bufs=8, space="PSUM") as psump,
        ):
            hs_b0s = [
                dramp.tile([cfg.H0R, 128], bf16, name=f"hsb0_{j}") for j in range(cfg.K)
            ]
            hs_b1s = [
                dramp.tile([cfg.H1R, 128], bf16, name=f"hsb1_{j}") for j in range(cfg.K)
            ]
            hs_f0s = [
                dramp.tile([cfg.NC * cfg.H0R, 128], bf16, addr_space="Shared", name=f"hsf0_{j}")
                for j in range(cfg.K)
            ]
            hs_f1s = [
                dramp.tile([cfg.NC * cfg.H1R, 128], bf16, addr_space="Shared", name=f"hsf1_{j}")
                for j in range(cfg.K)
            ]
            hs_b0_vs = [hb.rearrange("(n p) c -> p n c", p=128) for hb in hs_b0s]
            hs_b1_vs = [hb.rearrange("(n p) c -> p n c", p=128) for hb in hs_b1s]

            def hs_window_view(it, i):
                if i < cfg.H0W:
                    return hs_b0_vs[it][:, i, :]
                return hs_b1_vs[it][:, i - cfg.H0W, :]

            def chunk_src(it, qq):
                if qq < 2:
                    return hs_f0s[it][qq * cfg.CROWS0 : (qq + 1) * cfg.CROWS0, :]
                return hs_f1s[it][(qq - 2) * cfg.CROWS1 : (qq - 1) * cfg.CROWS1, :]

            out_v = out_d.rearrange("(n p) c -> p n c", p=128)

            # ---- resident constants ----
            w1_s = constp.tile([128, KIN, cfg.HD], f32)
            nc.sync.dma_start(w1_s[:], w1_d.rearrange("(k p) h -> p k h", p=128)[:, :, :])
            b1_s = constp.tile([cfg.HD, 1], f32)
            nc.sync.dma_start(b1_s[:], b1_d[:])
            w2_s = constp.tile([cfg.HD, cfg.OUT], f32)
            nc.sync.dma_start(w2_s[:], w2_d[:])
            b2_s = constp.tile([128, cfg.OUT], f32)
            nc.sync.dma_start(b2_s[:], b2_d[:])
            nsrc_s = constp.tile([128, cfg.NT], f32)
            nc.sync.dma_start(nsrc_s[:], nsrc_d[:])
            ndst_s = constp.tile([128, cfg.NT], f32)
            nc.sync.dma_start(ndst_s[:], ndst_d[:])
            zero512 = constp.tile([128, 512], bf16)
            nc.vector.memset(zero512[:], 0)
            alpha_h0 = constp.tile([128, cfg.NT, cfg.OUT], bf16)

            # ---- MLP encoder: h0 = relu(F @ W1 + b1) @ W2 + b2 ----
            ft_v = ft_d.rearrange("(k p) c -> p k c", p=128)
            for i in range(cfg.NT):
                ft_t = mlpio.tile([128, KIN, 128], f32, tag="ft")
                nc.sync.dma_start(ft_t[:], ft_v[:, :, i * 128 : (i + 1) * 128])
                p_gt = psump.tile([cfg.HD, 128], f32, tag="pp")
                for k in range(KIN):
                    nc.tensor.matmul(
                        p_gt[:],
                        w1_s[:, k, :],
                        ft_t[:, k, :],
                        start=(k == 0),
                        stop=(k == KIN - 1),
                    )
                gt = mlpio.tile([cfg.HD, 128], f32, tag="gt")
                nc.scalar.activation(
                    gt[:], p_gt[:], mybir.ActivationFunctionType.Relu, bias=b1_s[:, 0:1]
                )
                p_h0 = psump.tile([128, cfg.OUT], f32, tag="pp")
                nc.tensor.matmul(p_h0[:], gt[:], w2_s[:], start=True, stop=True)
                h0_t = mlpio.tile([128, cfg.OUT], f32, tag="h0")
                nc.vector.tensor_add(h0_t[:], p_h0[:], b2_s[:])
                nc.scalar.activation(
                    alpha_h0[:, i, :],
                    h0_t[:],
                    mybir.ActivationFunctionType.Copy,
                    scale=cfg.ALPHA,
                )
                hs_t = mlpio.tile([128, 128], bf16, tag="hs")
                nc.vector.memset(hs_t[:, cfg.OUT :], 0)
                nc.vector.tensor_scalar_mul(hs_t[:, 0 : cfg.OUT], h0_t[:], nsrc_s[:, i : i + 1])
                nc.sync.dma_start(hs_window_view(0, i), hs_t[:])

            # ---- K propagation steps ----
            for it in range(cfg.K):
                nc.gpsimd.collective_compute(
                    "AllGather",
                    mybir.AluOpType.bypass,
                    replica_groups=[list(range(cfg.NC))],
                    ins=[hs_b0s[it].opt()],
                    outs=[hs_f0s[it].opt()],
                )
                nc.gpsimd.collective_compute(
                    "AllGather",
                    mybir.AluOpType.bypass,
                    replica_groups=[list(range(cfg.NC))],
                    ins=[hs_b1s[it].opt()],
                    outs=[hs_f1s[it].opt()],
                )
                call_cursor = 0
                qcount = 0
                last = it == cfg.K - 1
                for bi, (ws, we, sgl) in enumerate(cfg.batches):
                    nbank = -(-(we - ws) // cfg.SG)
                    banks = [
                        psump.tile([128, 512], f32, tag="pp", name=f"pb{it}_{bi}_{j}")
                        for j in range(nbank)
                    ]
                    psum_map = {}
                    for ww in range(ws, we):
                        wi = ww - ws
                        psum_map[ww] = banks[wi // cfg.SG][
                            :, (wi % cfg.SG) * 64 : (wi % cfg.SG) * 64 + 64
                        ]
                    for bk in banks:
                        # start=True zeroes the whole 2KB zero-region (= one
                        # bank); do it once per bank so its windows accumulate
                        # with start=False.
                        nc.tensor.matmul(
                            bk[:], zero512[:, 0:128], zero512[:],
                            start=True, stop=False, skip_group_check=True,
                        )
                    while call_cursor < len(calls):
                        qq, t0, ntl = calls[call_cursor]
                        if tiles[t0][0] * cfg.SG >= we:
                            break
                        plo, pn = call_pairs[call_cursor]
                        call_cursor += 1
                        nidx = ntl * 128
                        gi = idxp.tile([128, cfg.CALL_TILES * 8], i16, tag="gi")
                        nc.sync.dma_start(
                            gi[:, : ntl * 8], gidx_d[:, t0 * 8 : (t0 + ntl) * 8]
                        )
                        msg = msgp.tile([128, cfg.CALL_TILES, 128], bf16, tag="msg")
                        nc.gpsimd.dma_gather(
                            msg[:, :ntl, :],
                            chunk_src(it, qq),
                            gi[:, : ntl * 8],
                            nidx,
                            nidx,
                            128,
                            single_packet=False,
                            queue_num=qcount % 4,
                        )
                        qcount += 1
                        oh = ohp.tile([128, cfg.CALL_TILES * 2, 128], bf16, tag="oh")
                        nc.sync.dma_start(oh[:, :pn, :], oh_v[:, plo : plo + pn, :])
                        for j in range(pn):
                            t, wv = pairs[plo + j]
                            nc.tensor.matmul(
                                psum_map[wv],
                                oh[:, j, :],
                                msg[:, t - t0, 0 : cfg.OUT],
                                start=False,
                                stop=False,
                                skip_group_check=True,
                            )
                    # epilogue for this batch
                    nw = we - ws
                    stg = stagep.tile(
                        [128, max_nw, cfg.OUT], f32, tag="stgf",
                        name=f"sf{it}_{bi}",
                    )
                    for ww in range(ws, we):
                        wi = ww - ws
                        t1 = epp.tile([128, cfg.OUT], f32, tag="t1")
                        if ppw[ww] == 0:
                            nc.vector.memset(t1[:], 0.0)
                        else:
                            nc.scalar.activation(
                                t1[:],
                                psum_map[ww],
                                mybir.ActivationFunctionType.Copy,
                                scale=ndst_s[:, ww : ww + 1],
                            )
                        if last:
                            nc.vector.tensor_add(
                                stg[:, wi, :], t1[:], alpha_h0[:, ww, :]
                            )
                        else:
                            t2 = epp.tile([128, cfg.OUT], f32, tag="t2")
                            nc.vector.tensor_add(t2[:], t1[:], alpha_h0[:, ww, :])
                            nc.vector.tensor_scalar_mul(
                                stg[:, wi, :], t2[:], nsrc_s[:, ww : ww + 1]
                            )
                    if last:
                        nc.sync.dma_start(out_v[:, ws:we, :], stg[:, :nw, :])
                    else:
                        stb = stagep.tile(
                            [128, max_nw, 128], bf16, tag="stgb",
                            name=f"sb{it}_{bi}",
                        )
                        nc.vector.memset(stb[:, :nw, cfg.OUT :], 0)
                        nc.scalar.activation(
                            stb[:, :nw, 0 : cfg.OUT],
                            stg[:, :nw, :],
                            mybir.ActivationFunctionType.Copy,
                        )
                        if we <= cfg.H0W:
                            nc.sync.dma_start(
                                hs_b0_vs[it + 1][:, ws:we, :], stb[:, :nw, :]
                            )
                        elif ws >= cfg.H0W:
                            nc.sync.dma_start(
                                hs_b1_vs[it + 1][:, ws - cfg.H0W : we - cfg.H0W, :],
                                stb[:, :nw, :],
                            )
                        else:
                            k0 = cfg.H0W - ws
                            nc.sync.dma_start(
                                hs_b0_vs[it + 1][:, ws : cfg.H0W, :], stb[:, :k0, :]
                            )
                            nc.sync.dma_start(
                                hs_b1_vs[it + 1][:, 0 : we - cfg.H0W, :],
                                stb[:, k0:nw, :],
                            )
    nc.compile()
    return nc


_CACHE = {}


def _get_compiled(cfg: Cfg, sched_sig, sched):
    key = (cfg, sched_sig)
    if key not in _CACHE:
        _CACHE[key] = _build_nc(cfg, sched)
    return _CACHE[key]


def kernel(features, edge_index, W1, b1, W2, b2):
    cfg = CFG
    features = np.asarray(features, dtype=np.float32)
    W1 = np.asarray(W1, dtype=np.float32)
    b1 = np.asarray(b1, dtype=np.float32)
    W2 = np.asarray(W2, dtype=np.float32)
    b2 = np.asarray(b2, dtype=np.float32)

    sched, per_core, norm_src, norm_dst = _preprocess(edge_index, cfg)
    sched_sig = (
        sched["TOT"],
        sched["NPAIR"],
        tuple(sched["calls"]),
        tuple(sched["pairs"]),
    )
    nc = _get_compiled(cfg, sched_sig, sched)

    b1c = b1.reshape(cfg.HD, 1)
    b2bc = np.ascontiguousarray(np.broadcast_to(b2[None, :], (128, cfg.OUT)))

    in_maps = []
    for c in range(cfg.NC):
        lo = c * cfg.NSH
        f = np.zeros((cfg.IN, cfg.NPAD), dtype=np.float32)
        f[:, : cfg.NSH] = features[lo : lo + cfg.NSH].T
        ns = np.ones(cfg.NPAD, dtype=np.float32)
        ns[: cfg.NSH] = norm_src[lo : lo + cfg.NSH]
        nd = np.ones(cfg.NPAD, dtype=np.float32)
        nd[: cfg.NSH] = norm_dst[lo : lo + cfg.NSH]
        gidx, oh = per_core[c]
        in_maps.append(
            {
                "ft": f,
                "w1": W1,
                "b1c": b1c,
                "w2": W2,
                "b2bc": b2bc,
                "nsrc": np.ascontiguousarray(ns.reshape(cfg.NT, 128).T),
                "ndst09": np.ascontiguousarray(
                    ((1.0 - cfg.ALPHA) * nd).reshape(cfg.NT, 128).T
                ),
                "gidx": gidx,
                "oh": oh,
            }
        )

    res = run_bass_kernel_spmd(nc, in_maps, list(range(cfg.NC)))
    out = np.concatenate(
        [res.results[c]["out"][: cfg.NSH] for c in range(cfg.NC)], axis=0
    )
    return out


# revision 12
# speedup vs baseline: 1.2553x; 1.2275x over previous
"""APPNP GNN kernel for 8 TRN2 NeuronCores (v2).

Strategy (vertex-cut / dst-partitioned):
  - Nodes row-sharded across 8 cores (12500 rows/core, padded to 12544).
  - MLP encoder data-parallel per core (features host-transposed).
  - K=10 propagation steps; each step:
      AllGather of norm-scaled h (bf16, rows padded to 256B),
      dma_gather of source rows for this core's in-edges (4 src chunks so
      row indices fit int16; tokens padded per (supergroup-of-8-windows,
      chunk) to a multiple of 128 -> ~2-4% padding),
      segment-sum via one-hot stationary matmuls accumulating in PSUM
      (128-dst-node windows; one-hots precomputed on host, streamed from
      DRAM), epilogue fuses norm_dst scale + alpha*h0 + norm_src scale.
  - dma_scatter_add is NOT used (HBM CCE RMW loses updates on duplicate
    indices).
"""

import sys

sys.path.insert(0, "/opt/trn_rl_repo")

import dataclasses
import numpy as np
import ml_dtypes

import concourse.bass as bass
import concourse.mybir as mybir
import concourse.tile as tile
from concourse import bacc
from concourse.bass_utils import run_bass_kernel_spmd

BF16 = ml_dtypes.bfloat16


@dataclasses.dataclass(frozen=True)
class Cfg:
    NC: int = 8          # cores
    N: int = 100000      # nodes
    IN: int = 512        # input feature dim
    HD: int = 64         # hidden dim
    OUT: int = 64        # output dim
    K: int = 10          # propagation steps
    ALPHA: float = 0.1
    NCHUNK: int = 4      # gather source chunks (int16 idx range)
    CALL_TILES: int = 32  # max tiles (128 tokens each) per dma_gather call
    SG: int = 8          # windows per supergroup (= windows per psum bank)
    BSG: int = 3         # supergroups per batch
    H0W: int = 48        # windows in AG half 0 (must be a batch boundary)

    @property
    def NSH(self):
        return self.N // self.NC

    @property
    def NT(self):
        return (self.NSH + 127) // 128  # windows per core

    @property
    def NPAD(self):
        return self.NT * 128

    @property
    def GROWS(self):
        return self.NC * self.NPAD

    @property
    def H0R(self):
        return self.H0W * 128

    @property
    def H1R(self):
        return self.NPAD - self.H0R

    @property
    def CROWS0(self):
        return self.NC * self.H0R // 2

    @property
    def CROWS1(self):
        return self.NC * self.H1R // 2

    @property
    def supergroups(self):
        out = []
        w = 0
        while w < self.NT:
            out.append((w, min(w + self.SG, self.NT)))
            w += self.SG
        return out

    @property
    def batches(self):
        """Batches of supergroups: list of (ws, we, [sg indices])."""
        sgs = self.supergroups
        out = []
        i = 0
        while i < len(sgs):
            j = min(i + self.BSG, len(sgs))
            if len(sgs) - j == 1:  # avoid lone trailing supergroup
                j = len(sgs)
            out.append((sgs[i][0], sgs[j - 1][1], list(range(i, j))))
            i = j
        return out


CFG = Cfg()


def _preprocess(edge_index, cfg: Cfg):
    src = np.asarray(edge_index[0], dtype=np.int64)
    dst = np.asarray(edge_index[1], dtype=np.int64)

    deg_out = np.bincount(src, minlength=cfg.N).astype(np.float32)
    deg_in = np.bincount(dst, minlength=cfg.N).astype(np.float32)
    norm_src = 1.0 / np.sqrt(np.clip(deg_out, 1.0, None))
    norm_dst = 1.0 / np.sqrt(np.clip(deg_in, 1.0, None))

    core = dst // cfg.NSH
    dst_local = dst - core * cfg.NSH
    w = dst_local // 128
    slot = dst_local % 128
    sg = w // cfg.SG
    d = src // cfg.NSH
    j = src % cfg.NSH
    h1 = j >= cfg.H0R
    g0 = d * cfg.H0R + j
    g1 = d * cfg.H1R + (j - cfg.H0R)
    q = np.where(h1, 2 + g1 // cfg.CROWS1, g0 // cfg.CROWS0)
    lidx = np.where(h1, g1 % cfg.CROWS1, g0 % cfg.CROWS0)

    NSG = len(cfg.supergroups)
    key = (core * NSG + sg) * cfg.NCHUNK + q
    counts = np.bincount(key, minlength=cfg.NC * NSG * cfg.NCHUNK).reshape(
        cfg.NC, NSG, cfg.NCHUNK
    )
    P = -(-counts.max(axis=0) // 128)  # [NSG, NCHUNK] tiles per (sg, q)

    # static tile stream: per batch, per chunk, supergroups ascending
    tiles = []          # (sg, q)
    calls = []          # (q, tile_off, ntiles)
    grp_tile0 = {}      # (sg, q) -> first tile index
    for (_, _, sgl) in cfg.batches:
        for qq in range(cfg.NCHUNK):
            group = []
            for si in sgl:
                grp_tile0[(si, qq)] = len(tiles) + len(group)
                group += [(si, qq)] * int(P[si, qq])
            off = 0
            while off < len(group):
                n = min(cfg.CALL_TILES, len(group) - off)
                calls.append((qq, len(tiles) + off, n))
                off += n
            tiles.extend(group)
    TOT = len(tiles)

    # per-core token placement: edges sorted by (core, sg, q, w, slot)
    sort_key = (((core * NSG + sg) * cfg.NCHUNK + q) * cfg.NT + w) * 128 + slot
    order = np.argsort(sort_key, kind="stable")
    s_core, s_sg, s_q = core[order], sg[order], q[order]
    s_w, s_slot, s_lidx = w[order], slot[order], lidx[order]

    gkey = (s_core * NSG + s_sg) * cfg.NCHUNK + s_q
    gstarts = np.searchsorted(gkey, np.arange(cfg.NC * NSG * cfg.NCHUNK))
    gends = np.searchsorted(gkey, np.arange(cfg.NC * NSG * cfg.NCHUNK) + 1)

    tile_windows = [set() for _ in range(TOT)]
    core_tok = []  # per core: (tile, part, w, slot, lidx) arrays
    for c in range(cfg.NC):
        tl_l, pp_l, ww_l, sl_l, li_l = [], [], [], [], []
        for si in range(NSG):
            for qq in range(cfg.NCHUNK):
                gi = (c * NSG + si) * cfg.NCHUNK + qq
                a, b = gstarts[gi], gends[gi]
                if b == a:
                    continue
                t0 = grp_tile0[(si, qq)]
                j = np.arange(b - a)
                tl_l.append(t0 + j // 128)
                pp_l.append(j % 128)
                ww_l.append(s_w[a:b])
                sl_l.append(s_slot[a:b])
                li_l.append(s_lidx[a:b])
        tl = np.concatenate(tl_l)
        ww = np.concatenate(ww_l)
        core_tok.append(
            (tl, np.concatenate(pp_l), ww, np.concatenate(sl_l), np.concatenate(li_l))
        )
        # distinct (tile, window) incidences for this core
        tw = np.unique(tl.astype(np.int64) * cfg.NT + ww)
        for v in tw:
            tile_windows[int(v) // cfg.NT].add(int(v) % cfg.NT)

    pairs = []
    pair_idx = {}
    for t in range(TOT):
        for wv in sorted(tile_windows[t]):
            pair_idx[(t, wv)] = len(pairs)
            pairs.append((t, wv))
    NPAIR = len(pairs)

    call_pairs = []
    for (qq, t0, nt) in calls:
        idxs = [
            pair_idx[(t, wv)] for t in range(t0, t0 + nt) for wv in tile_windows[t]
        ]
        call_pairs.append((min(idxs) if idxs else 0, len(idxs)))

    per_core = []
    for c in range(cfg.NC):
        tl, pp, ww, sl, li = core_tok[c]
        g_tok = np.zeros(TOT * 128, dtype=np.int16)
        g_tok[tl * 128 + pp] = li.astype(np.int16)
        cols = []
        for (qq, t0, nt) in calls:
            seg = g_tok[t0 * 128 : (t0 + nt) * 128]
            cols.append(np.tile(seg.reshape(-1, 16).T, (8, 1)))
        gidx = np.ascontiguousarray(np.concatenate(cols, axis=1)).astype(np.int16)

        pj = np.array(
            [pair_idx[(int(t), int(wv))] for t, wv in zip(tl, ww)], dtype=np.int64
        )
        oh = np.zeros((128, NPAIR * 128), dtype=BF16)
        oh[pp, pj * 128 + sl] = BF16(1.0)
        per_core.append((gidx, oh))

    sched = {
        "P": P,
        "tiles": tiles,
        "calls": calls,
        "call_pairs": call_pairs,
        "pairs": pairs,
        "TOT": TOT,
        "NPAIR": NPAIR,
        "pairs_per_w": np.bincount(
            np.array([wv for (_, wv) in pairs], dtype=np.int64), minlength=cfg.NT
        ),
    }
    return sched, per_core, norm_src, norm_dst


def _build_nc(cfg: Cfg, sched):
    f32 = mybir.dt.float32
    bf16 = mybir.dt.bfloat16
    i16 = mybir.dt.int16
    TOT = sched["TOT"]
    tiles = sched["tiles"]
    calls = sched["calls"]
    call_pairs = sched["call_pairs"]
    pairs = sched["pairs"]
    NPAIR = sched["NPAIR"]
    ppw = sched["pairs_per_w"]

    nc = bacc.Bacc(
        "TRN2",
        target_bir_lowering=False,
        debug=False,
        num_devices=cfg.NC,
        num_swdge_queues=4,
        dynamic_dma_scratch_size=16384,
    )
    ft_d = nc.dram_tensor("ft", [cfg.IN, cfg.NPAD], f32, kind="ExternalInput")
    w1_d = nc.dram_tensor("w1", [cfg.IN, cfg.HD], f32, kind="ExternalInput")
    b1_d = nc.dram_tensor("b1c", [cfg.HD, 1], f32, kind="ExternalInput")
    w2_d = nc.dram_tensor("w2", [cfg.HD, cfg.OUT], f32, kind="ExternalInput")
    b2_d = nc.dram_tensor("b2bc", [128, cfg.OUT], f32, kind="ExternalInput")
    nsrc_d = nc.dram_tensor("nsrc", [128, cfg.NT], f32, kind="ExternalInput")
    ndst_d = nc.dram_tensor("ndst09", [128, cfg.NT], f32, kind="ExternalInput")
    gidx_d = nc.dram_tensor("gidx", [128, TOT * 8], i16, kind="ExternalInput")
    oh_d = nc.dram_tensor("oh", [128, NPAIR * 128], bf16, kind="ExternalInput")
    out_d = nc.dram_tensor("out", [cfg.NPAD, cfg.OUT], f32, kind="ExternalOutput")

    KIN = cfg.IN // 128
    oh_v = oh_d.rearrange("p (n i) -> p n i", i=128)
    max_nw = max(we - ws for (ws, we, _) in cfg.batches)

    with tile.TileContext(nc) as tc:
        with (
            tc.tile_pool(name="const", bufs=1) as constp,
            tc.tile_pool(name="dram", bufs=1, space="DRAM") as dramp,
            tc.tile_pool(name="mlpio", bufs=3) as mlpio,
            tc.tile_pool(name="msgp", bufs=4) as msgp,
            tc.tile_pool(name="ohp", bufs=3) as ohp,
            tc.tile_pool(name="stagep", bufs=2) as stagep,
            tc.tile_pool(name="epp", bufs=8) as epp,
            tc.tile_pool(name="psum", bufs=8, space="PSUM") as psump,
        ):
            hs_b0s = [
                dramp.tile([cfg.H0R, 128], bf16, name=f"hsb0_{j}") for j in range(cfg.K)
            ]
            hs_b1s = [
                dramp.tile([cfg.H1R, 128], bf16, name=f"hsb1_{j}") for j in range(cfg.K)
            ]
            hs_f0s = [
                dramp.tile([cfg.NC * cfg.H0R, 128], bf16, addr_space="Shared", name=f"hsf0_{j}")
                for j in range(cfg.K)
            ]
            hs_f1s = [
                dramp.tile([cfg.NC * cfg.H1R, 128], bf16, addr_space="Shared", name=f"hsf1_{j}")
                for j in range(cfg.K)
            ]
            hs_b0_vs = [hb.rearrange("(n p) c -> p n c", p=128) for hb in hs_b0s]
            hs_b1_vs = [hb.rearrange("(n p) c -> p n c", p=128) for hb in hs_b1s]

            def hs_window_view(it, i):
                if i < cfg.H0W:
                    return hs_b0_vs[it][:, i, :]
                return hs_b1_vs[it][:, i - cfg.H0W, :]

            def chunk_src(it, qq):
                if qq < 2:
                    return hs_f0s[it][qq * cfg.CROWS0 : (qq + 1) * cfg.CROWS0, :]
                return hs_f1s[it][(qq - 2) * cfg.CROWS1 : (qq - 1) * cfg.CROWS1, :]

            out_v = out_d.rearrange("(n p) c -> p n c", p=128)

            # ---- resident constants ----
            w1_s = constp.tile([128, KIN, cfg.HD], f32)
            nc.sync.dma_start(w1_s[:], w1_d.rearrange("(k p) h -> p k h", p=128)[:, :, :])
            b1_s = constp.tile([cfg.HD, 1], f32)
            nc.sync.dma_start(b1_s[:], b1_d[:])
            w2_s = constp.tile([cfg.HD, cfg.OUT], f32)
            nc.sync.dma_start(w2_s[:], w2_d[:])
            b2_s = constp.tile([128, cfg.OUT], f32)
            nc.sync.dma_start(b2_s[:], b2_d[:])
            nsrc_s = constp.tile([128, cfg.NT], f32)
            nc.sync.dma_start(nsrc_s[:], nsrc_d[:])
            ndst_s = constp.tile([128, cfg.NT], f32)
            nc.sync.dma_start(ndst_s[:], ndst_d[:])
            gidx_s = constp.tile([128, TOT * 8], i16)
            nc.sync.dma_start(gidx_s[:], gidx_d[:])
            zero512 = constp.tile([128, 512], bf16)
            nc.vector.memset(zero512[:], 0)
            alpha_h0 = constp.tile([128, cfg.NT, cfg.OUT], bf16)

            # ---- MLP encoder: h0 = relu(F @ W1 + b1) @ W2 + b2 ----
            ft_v = ft_d.rearrange("(k p) c -> p k c", p=128)
            for i in range(cfg.NT):
                ft_t = mlpio.tile([128, KIN, 128], f32, tag="ft")
                nc.sync.dma_start(ft_t[:], ft_v[:, :, i * 128 : (i + 1) * 128])
                p_gt = psump.tile([cfg.HD, 128], f32, tag="pp")
                for k in range(KIN):
                    nc.tensor.matmul(
                        p_gt[:],
                        w1_s[:, k, :],
                        ft_t[:, k, :],
                        start=(k == 0),
                        stop=(k == KIN - 1),
                    )
                gt = mlpio.tile([cfg.HD, 128], f32, tag="gt")
                nc.scalar.activation(
                    gt[:], p_gt[:], mybir.ActivationFunctionType.Relu, bias=b1_s[:, 0:1]
                )
                p_h0 = psump.tile([128, cfg.OUT], f32, tag="pp")
                nc.tensor.matmul(p_h0[:], gt[:], w2_s[:], start=True, stop=True)
                h0_t = mlpio.tile([128, cfg.OUT], f32, tag="h0")
                nc.vector.tensor_add(h0_t[:], p_h0[:], b2_s[:])
                nc.scalar.activation(
                    alpha_h0[:, i, :],
                    h0_t[:],
                    mybir.ActivationFunctionType.Copy,
                    scale=cfg.ALPHA,
                )
                hs_t = mlpio.tile([128, 128], bf16, tag="hs")
                nc.vector.memset(hs_t[:, cfg.OUT :], 0)
                nc.vector.tensor_scalar_mul(hs_t[:, 0 : cfg.OUT], h0_t[:], nsrc_s[:, i : i + 1])
                nc.sync.dma_start(hs_window_view(0, i), hs_t[:])

            # ---- K propagation steps ----
            for it in range(cfg.K):
                nc.gpsimd.collective_compute(
                    "AllGather",
                    mybir.AluOpType.bypass,
                    replica_groups=[list(range(cfg.NC))],
                    ins=[hs_b0s[it].opt()],
                    outs=[hs_f0s[it].opt()],
                )
                nc.gpsimd.collective_compute(
                    "AllGather",
                    mybir.AluOpType.bypass,
                    replica_groups=[list(range(cfg.NC))],
                    ins=[hs_b1s[it].opt()],
                    outs=[hs_f1s[it].opt()],
                )
                call_cursor = 0
                qcount = 0
                last = it == cfg.K - 1
                for bi, (ws, we, sgl) in enumerate(cfg.batches):
                    nbank = -(-(we - ws) // cfg.SG)
                    banks = [
                        psump.tile([128, 512], f32, tag="pp", name=f"pb{it}_{bi}_{j}")
                        for j in range(nbank)
                    ]
                    psum_map = {}
                    for ww in range(ws, we):
                        wi = ww - ws
                        psum_map[ww] = banks[wi // cfg.SG][
                            :, (wi % cfg.SG) * 64 : (wi % cfg.SG) * 64 + 64
                        ]
                    for bk in banks:
                        # start=True zeroes the whole 2KB zero-region (= one
                        # bank); do it once per bank so its windows accumulate
                        # with start=False.
                        nc.tensor.matmul(
                            bk[:], zero512[:, 0:128], zero512[:],
                            start=True, stop=False, skip_group_check=True,
                        )
                    while call_cursor < len(calls):
                        qq, t0, ntl = calls[call_cursor]
                        if tiles[t0][0] * cfg.SG >= we:
                            break
                        plo, pn = call_pairs[call_cursor]
                        call_cursor += 1
                        nidx = ntl * 128
                        msg = msgp.tile([128, cfg.CALL_TILES, 128], bf16, tag="msg")
                        nc.gpsimd.dma_gather(
                            msg[:, :ntl, :],
                            chunk_src(it, qq),
                            gidx_s[:, t0 * 8 : (t0 + ntl) * 8],
                            nidx,
                            nidx,
                            128,
                            single_packet=False,
                            queue_num=qcount % 4,
                        )
                        qcount += 1
                        oh = ohp.tile([128, cfg.CALL_TILES * 2, 128], bf16, tag="oh")
                        nc.sync.dma_start(oh[:, :pn, :], oh_v[:, plo : plo + pn, :])
                        for j in range(pn):
                            t, wv = pairs[plo + j]
                            nc.tensor.matmul(
                                psum_map[wv],
                                oh[:, j, :],
                                msg[:, t - t0, 0 : cfg.OUT],
                                start=False,
                                stop=False,
                                skip_group_check=True,
                            )
                    # epilogue for this batch
                    nw = we - ws
                    stg = stagep.tile(
                        [128, max_nw, cfg.OUT], f32, tag="stgf",
                        name=f"sf{it}_{bi}",
                    )
                    for ww in range(ws, we):
                        wi = ww - ws
                        t1 = epp.tile([128, cfg.OUT], f32, tag="t1")
                        if ppw[ww] == 0:
                            nc.vector.memset(t1[:], 0.0)
                        else:
                            nc.scalar.activation(
                                t1[:],
                                psum_map[ww],
                                mybir.ActivationFunctionType.Copy,
                                scale=ndst_s[:, ww : ww + 1],
                            )
                        if last:
                            nc.vector.tensor_add(
                                stg[:, wi, :], t1[:], alpha_h0[:, ww, :]
                            )
                        else:
                            t2 = epp.tile([128, cfg.OUT], f32, tag="t2")
                            nc.vector.tensor_add(t2[:], t1[:], alpha_h0[:, ww, :])
                            nc.vector.tensor_scalar_mul(
                                stg[:, wi, :], t2[:], nsrc_s[:, ww : ww + 1]
                            )
                    if last:
                        nc.sync.dma_start(out_v[:, ws:we, :], stg[:, :nw, :])
                    else:
                        stb = stagep.tile(
                            [128, max_nw, 128], bf16, tag="stgb",
                            name=f"sb{it}_{bi}",
                        )
                        nc.vector.memset(stb[:, :nw, cfg.OUT :], 0)
                        nc.scalar.activation(
                            stb[:, :nw, 0 : cfg.OUT],
                            stg[:, :nw, :],
                            mybir.ActivationFunctionType.Copy,
                        )
                        if we <= cfg.H0W:
                            nc.sync.dma_start(
                                hs_b0_vs[it + 1][:, ws:we, :], stb[:, :nw, :]
                            )
                        elif ws >= cfg.H0W:
                            nc.sync.dma_start(
                                hs_b1_vs[it + 1][:, ws - cfg.H0W : we - cfg.H0W, :],
                                stb[:, :nw, :],
                            )
                        else:
                            k0 = cfg.H0W - ws
                            nc.sync.dma_start(
                                hs_b0_vs[it + 1][:, ws : cfg.H0W, :], stb[:, :k0, :]
                            )
                            nc.sync.dma_start(
                                hs_b1_vs[it + 1][:, 0 : we - cfg.H0W, :],
                                stb[:, k0:nw, :],
                            )
    nc.compile()
    return nc


_CACHE = {}


def _get_compiled(cfg: Cfg, sched_sig, sched):
    key = (cfg, sched_sig)
    if key not in _CACHE:
        _CACHE[key] = _build_nc(cfg, sched)
    return _CACHE[key]


def kernel(features, edge_index, W1, b1, W2, b2):
    cfg = CFG
    features = np.asarray(features, dtype=np.float32)
    W1 = np.asarray(W1, dtype=np.float32)
    b1 = np.asarray(b1, dtype=np.float32)
    W2 = np.asarray(W2, dtype=np.float32)
    b2 = np.asarray(b2, dtype=np.float32)

    sched, per_core, norm_src, norm_dst = _preprocess(edge_index, cfg)
    sched_sig = (
        sched["TOT"],
        sched["NPAIR"],
        tuple(sched["calls"]),
        tuple(sched["pairs"]),
    )
    nc = _get_compiled(cfg, sched_sig, sched)

    b1c = b1.reshape(cfg.HD, 1)
    b2bc = np.ascontiguousarray(np.broadcast_to(b2[None, :], (128, cfg.OUT)))

    in_maps = []
    for c in range(cfg.NC):
        lo = c * cfg.NSH
        f = np.zeros((cfg.IN, cfg.NPAD), dtype=np.float32)
        f[:, : cfg.NSH] = features[lo : lo + cfg.NSH].T
        ns = np.ones(cfg.NPAD, dtype=np.float32)
        ns[: cfg.NSH] = norm_src[lo : lo + cfg.NSH]
        nd = np.ones(cfg.NPAD, dtype=np.float32)
        nd[: cfg.NSH] = norm_dst[lo : lo + cfg.NSH]
        gidx, oh = per_core[c]
        in_maps.append(
            {
                "ft": f,
                "w1": W1,
                "b1c": b1c,
                "w2": W2,
                "b2bc": b2bc,
                "nsrc": np.ascontiguousarray(ns.reshape(cfg.NT, 128).T),
                "ndst09": np.ascontiguousarray(
                    ((1.0 - cfg.ALPHA) * nd).reshape(cfg.NT, 128).T
                ),
                "gidx": gidx,
                "oh": oh,
            }
        )

    res = run_bass_kernel_spmd(nc, in_maps, list(range(cfg.NC)))
    out = np.concatenate(
        [res.results[c]["out"][: cfg.NSH] for c in range(cfg.NC)], axis=0
    )
    return out
